# revision 2
# baseline (speedup 1.0000x reference)
"""Trainium2 Bass kernel for nn_LinearRNN (B=16, T=4096, D_in=256, H=512, D_out=256).

  xp = x @ W_in.T                       [B, T, H]
  h_t = xp_t + h_{t-1} @ W_h.T          (W_h is diagonal -> elementwise scan)
  out = hs @ W_out.T                    [B, T, D_out]

Strategy: batch data-parallel over 8 cores (2 batch rows per core). Per core:
  - host pre-transposes x to [b, d, t]; weights pre-transposed likewise.
  - matmul1 on TensorE produces xp tiles [h=128, t=512] in PSUM,
  - VectorE tensor_tensor_scan runs the recurrence along the free (t) axis
    with the per-h decay broadcast from a [128,1] column, carry chained
    across t-chunks via the previous tile's last column,
  - matmul2 on TensorE contracts h back to d_out, ScalarE copies PSUM->SBUF,
  - output [b, o, t] DMAs back and the host transposes to [b, t, o].

Modes:
  'hl8'  : matmul1 via fp8e4m3 hi/lo DoubleRow (3 instrs per 256-contraction
           at 0.5 cyc/row = 1.5 cyc vs 2.0 for bf16 -> 25% faster mm1), scan
           writes bf16 hs, matmul2 in bf16, bf16 output. PE warmup matmuls
           ramp the clock during the initial DMA fill.
  'f32r' : fp32 storage, PE runs reduced-precision single-pass (1 cyc/row)
  'bf16' : x/weights/hs cast to bf16 (halves input DMA)
  'f32'  : exact fp32 matmuls (4 cyc/row on PE)
"""
from contextlib import ExitStack

import numpy as np

import concourse.bass as bass
import concourse.mybir as mybir
import concourse.tile as tile
from concourse import bacc
from concourse.bass_utils import run_bass_kernel_spmd

B, T, D_IN, HID, D_OUT = 16, 4096, 256, 512, 256
NCORES = 8
BPC = B // NCORES          # batch rows per core
TC = 512                   # t-chunk (PSUM bank = 512 fp32)
NCH = T // TC
ND = D_IN // 128           # 2  d-blocks
NH = HID // 128            # 4  h-blocks
NO = D_OUT // 128          # 2  o-blocks

SWI = 32.0                 # hl8: scale on W_in (out scaled by SWI, host divides)

MODE_DEFAULT = "hl8"

# schedule/tuning knobs (read by _build; cache key includes them)
CFG = dict(sched="pipe1x", xp_bufs=4, op_bufs=4, hs_bufs=16,
           x_piece=1024, out_piece=1024, wu_n=10)

_cache: dict = {}


# ---------------------------------------------------------------- hl8 build --

def _build_hl8() -> bass.Bass:
    f32 = mybir.dt.float32
    bf = mybir.dt.bfloat16
    fp8 = mybir.dt.float8e4
    DR = mybir.MatmulPerfMode.DoubleRow

    nc = bacc.Bacc(None, target_bir_lowering=False)

    # x planes per batch row: [128, 4, T]: 0=hi(d=p) 1=lo(d=p) 2=hi(d=128+p) 3=lo(d=128+p)
    xq = nc.declare_dram_parameter("xq", [BPC, 128, 4, T], fp8, isOutput=False)
    wa = nc.declare_dram_parameter("wa", [128, 2, HID], fp8, isOutput=False)
    wb = nc.declare_dram_parameter("wb", [128, 2, HID], fp8, isOutput=False)
    wc = nc.declare_dram_parameter("wc", [128, 2, HID], fp8, isOutput=False)
    w_outT = nc.declare_dram_parameter("w_outT", [HID, D_OUT], bf, isOutput=False)
    dcols = nc.declare_dram_parameter("dcols", [128, NH], f32, isOutput=False)
    out = nc.declare_dram_parameter("out", [BPC, D_OUT, T], bf, isOutput=True)

    with tile.TileContext(nc) as tc, ExitStack() as ctx:
        const_pool = ctx.enter_context(tc.tile_pool(name="const", bufs=1))
        x_pool = ctx.enter_context(tc.tile_pool(name="xt", bufs=BPC))
        o_pool = ctx.enter_context(tc.tile_pool(name="ot", bufs=6))
        hs_pool = ctx.enter_context(tc.tile_pool(name="hs", bufs=CFG["hs_bufs"]))
        xp_psum = ctx.enter_context(
            tc.tile_pool(name="xp", bufs=CFG["xp_bufs"], space=bass.MemorySpace.PSUM))
        op_psum = ctx.enter_context(
            tc.tile_pool(name="op", bufs=CFG["op_bufs"], space=bass.MemorySpace.PSUM))

        # --- PE warmup: ramp the tensor-engine clock while input DMAs land.
        # Matmuls on a zeroed SBUF tile into a rotating PSUM buf; no DMA deps,
        # so they start at t~0 and keep PE continuously busy into real work.
        if CFG.get("wu_n", 0):
            wu = const_pool.tile([128, TC], bf, tag="wu")
            nc.gpsimd.memset(wu[:], 0.0)
            wu_ps = xp_psum.tile([128, TC], f32, name="wu", tag="xp")
            for _ in range(CFG["wu_n"]):
                nc.tensor.matmul(wu_ps[:], wu[:, 0:128], wu[:],
                                 start=True, stop=True)

        # --- DMAs: mm1 weights first (small), then first x piece, then rest.
        XP_LEN = CFG["x_piece"]
        xt = {}
        for b in range(BPC):
            xt[b] = x_pool.tile([128, 4, T], fp8, name="xt", tag="xt")

        def load_x(b, piece):
            psl = slice(piece * XP_LEN, (piece + 1) * XP_LEN)
            nc.sync.dma_start(xt[b][:, :, psl], xq[b, :, :, psl])

        wat = const_pool.tile([128, 2, HID], fp8, tag="wa")
        nc.sync.dma_start(wat[:], wa[:])
        wbt = const_pool.tile([128, 2, HID], fp8, tag="wb")
        nc.sync.dma_start(wbt[:], wb[:])
        wct = const_pool.tile([128, 2, HID], fp8, tag="wc")
        nc.sync.dma_start(wct[:], wc[:])
        load_x(0, 0)
        wo = []
        for hblk in range(NH):
            w = const_pool.tile([128, D_OUT], bf, tag=f"wo{hblk}")
            nc.sync.dma_start(w[:], w_outT[hblk * 128:(hblk + 1) * 128, :])
            wo.append(w)
        dc = const_pool.tile([128, NH], f32, tag="dc")
        nc.sync.dma_start(dc[:], dcols[:])
        for piece in range(1, T // XP_LEN):
            load_x(0, piece)
        for b in range(1, BPC):
            for piece in range(T // XP_LEN):
                load_x(b, piece)

        OP = CFG["out_piece"]
        ot = {}
        prev_hs = {}

        def stage1(b, ic):
            """hi/lo-fp8 DoubleRow matmul1 + scan for one (batch, chunk)."""
            for hblk in range(NH):
                hsl = slice(hblk * 128, (hblk + 1) * 128)
                xp = xp_psum.tile([128, TC], f32, name="xp", tag="xp")
                for c in range(TC // 256):
                    tsl = slice(ic * TC + c * 256, ic * TC + (c + 1) * 256)
                    osl = slice(c * 256, (c + 1) * 256)
                    nc.tensor.matmul(xp[:, osl], wat[:, :, hsl],
                                     xt[b][:, 0:2, tsl],
                                     start=True, stop=False, perf_mode=DR)
                    nc.tensor.matmul(xp[:, osl], wbt[:, :, hsl],
                                     xt[b][:, 2:4, tsl],
                                     start=False, stop=False, perf_mode=DR)
                    nc.tensor.matmul(xp[:, osl], wct[:, :, hsl],
                                     xt[b][:, 0:4:2, tsl],
                                     start=False, stop=True, perf_mode=DR)
                hs = hs_pool.tile([128, TC], bf, name="hs", tag="hs")
                init = (0.0 if ic == 0
                        else prev_hs[(b, ic - 1, hblk)][:, TC - 1:TC])
                nc.vector.tensor_tensor_scan(
                    hs[:], dc[:, hblk:hblk + 1].to_broadcast((128, TC)),
                    xp[:], init,
                    op0=mybir.AluOpType.mult, op1=mybir.AluOpType.add)
                prev_hs[(b, ic, hblk)] = hs

        def stage2(b, ic):
            """bf16 matmul2 + PSUM->SBUF bf16 copy (+ out DMA)."""
            q, csl = divmod(ic * TC, OP)
            for oblk in range(NO):
                op = op_psum.tile([128, TC], f32, name="op", tag="op")
                for hblk in range(NH):
                    nc.tensor.matmul(
                        op[:],
                        wo[hblk][:, oblk * 128:(oblk + 1) * 128],
                        prev_hs[(b, ic, hblk)][:],
                        start=(hblk == 0), stop=(hblk == NH - 1))
                if csl == 0:
                    ot[(b, oblk)] = o_pool.tile([128, OP], bf,
                                                name="ot", tag="ot")
                nc.scalar.copy(ot[(b, oblk)][:, csl:csl + TC], op[:])
                if csl + TC == OP:
                    nc.sync.dma_start(
                        out[b, oblk * 128:(oblk + 1) * 128,
                            q * OP:(q + 1) * OP],
                        ot[(b, oblk)][:])

        _schedule(stage1, stage2)

    nc.compile()
    return nc


def _schedule(stage1, stage2):
    sched = CFG.get("sched", "pipe1x")
    if sched == "interleave":
        for ic in range(NCH):
            for b in range(BPC):
                stage1(b, ic)
            for b in range(BPC):
                stage2(b, ic)
    elif sched == "pipe1":
        for b in range(BPC):
            stage1(b, 0)
            for ic in range(NCH - 1):
                stage1(b, ic + 1)
                stage2(b, ic)
            stage2(b, NCH - 1)
    elif sched == "pipe1x":
        order = [(b, ic) for b in range(BPC) for ic in range(NCH)]
        stage1(*order[0])
        for k in range(len(order) - 1):
            stage1(*order[k + 1])
            stage2(*order[k])
        stage2(*order[-1])
    else:
        for b in range(BPC):
            for ic in range(NCH):
                stage1(b, ic)
                stage2(b, ic)


# ------------------------------------------------------------- legacy build --

def _build(mode: str) -> bass.Bass:
    if mode == "hl8":
        return _build_hl8()
    f32 = mybir.dt.float32
    dt_in = {"bf16": mybir.dt.bfloat16, "f32r": mybir.dt.float32r}.get(mode, f32)
    dt_hs = dt_in

    nc = bacc.Bacc(None, target_bir_lowering=False)

    xT = nc.declare_dram_parameter("xT", [BPC, D_IN, T], dt_in, isOutput=False)
    w_inT = nc.declare_dram_parameter("w_inT", [D_IN, HID], dt_in, isOutput=False)
    w_outT = nc.declare_dram_parameter("w_outT", [HID, D_OUT], dt_in, isOutput=False)
    dcols = nc.declare_dram_parameter("dcols", [128, NH], f32, isOutput=False)
    out = nc.declare_dram_parameter("out", [BPC, D_OUT, T], f32, isOutput=True)

    with tile.TileContext(nc) as tc, ExitStack() as ctx:
        const_pool = ctx.enter_context(tc.tile_pool(name="const", bufs=1))
        x_pool = ctx.enter_context(tc.tile_pool(name="xt", bufs=BPC * ND))
        o_pool = ctx.enter_context(tc.tile_pool(name="ot", bufs=8))
        hs_pool = ctx.enter_context(tc.tile_pool(name="hs", bufs=CFG["hs_bufs"]))
        xp_psum = ctx.enter_context(
            tc.tile_pool(name="xp", bufs=CFG["xp_bufs"], space=bass.MemorySpace.PSUM))
        op_psum = ctx.enter_context(
            tc.tile_pool(name="op", bufs=CFG["op_bufs"], space=bass.MemorySpace.PSUM))

        XP_LEN = CFG["x_piece"]
        xt = {}
        for b in range(BPC):
            for dblk in range(ND):
                xt[(b, dblk)] = x_pool.tile([128, T], dt_in, name="xt", tag="xt")

        def load_x(b, dblk, piece):
            psl = slice(piece * XP_LEN, (piece + 1) * XP_LEN)
            nc.sync.dma_start(xt[(b, dblk)][:, psl],
                              xT[b, dblk * 128:(dblk + 1) * 128, psl])

        for dblk in range(ND):
            load_x(0, dblk, 0)
        wi = []
        for dblk in range(ND):
            w = const_pool.tile([128, HID], dt_in, tag=f"wi{dblk}")
            nc.sync.dma_start(w[:], w_inT[dblk * 128:(dblk + 1) * 128, :])
            wi.append(w)
        wo = []
        for hblk in range(NH):
            w = const_pool.tile([128, D_OUT], dt_in, tag=f"wo{hblk}")
            nc.sync.dma_start(w[:], w_outT[hblk * 128:(hblk + 1) * 128, :])
            wo.append(w)
        dc = const_pool.tile([128, NH], f32, tag="dc")
        nc.sync.dma_start(dc[:], dcols[:])
        for piece in range(1, T // XP_LEN):
            for dblk in range(ND):
                load_x(0, dblk, piece)
        for b in range(1, BPC):
            for piece in range(T // XP_LEN):
                for dblk in range(ND):
                    load_x(b, dblk, piece)

        OP = CFG["out_piece"]
        ot = {}
        prev_hs = {}

        def stage1(b, ic):
            tsl = slice(ic * TC, (ic + 1) * TC)
            for hblk in range(NH):
                xp = xp_psum.tile([128, TC], mybir.dt.float32, name="xp", tag="xp")
                for dblk in range(ND):
                    nc.tensor.matmul(
                        xp[:],
                        wi[dblk][:, hblk * 128:(hblk + 1) * 128],
                        xt[(b, dblk)][:, tsl],
                        start=(dblk == 0), stop=(dblk == ND - 1))
                hs = hs_pool.tile([128, TC], dt_hs, name="hs", tag="hs")
                init = (0.0 if ic == 0
                        else prev_hs[(b, ic - 1, hblk)][:, TC - 1:TC])
                nc.vector.tensor_tensor_scan(
                    hs[:], dc[:, hblk:hblk + 1].to_broadcast((128, TC)),
                    xp[:], init,
                    op0=mybir.AluOpType.mult, op1=mybir.AluOpType.add)
                prev_hs[(b, ic, hblk)] = hs

        def stage2(b, ic):
            q, csl = divmod(ic * TC, OP)
            for oblk in range(NO):
                op = op_psum.tile([128, TC], mybir.dt.float32, name="op", tag="op")
                for hblk in range(NH):
                    nc.tensor.matmul(
                        op[:],
                        wo[hblk][:, oblk * 128:(oblk + 1) * 128],
                        prev_hs[(b, ic, hblk)][:],
                        start=(hblk == 0), stop=(hblk == NH - 1))
                if csl == 0:
                    ot[(b, oblk)] = o_pool.tile([128, OP], mybir.dt.float32,
                                                name="ot", tag="ot")
                nc.scalar.copy(ot[(b, oblk)][:, csl:csl + TC], op[:])
                if csl + TC == OP:
                    nc.sync.dma_start(
                        out[b, oblk * 128:(oblk + 1) * 128,
                            q * OP:(q + 1) * OP],
                        ot[(b, oblk)][:])

        _schedule(stage1, stage2)

    nc.compile()
    return nc


# -------------------------------------------------------------- host side ----

def _prep_inputs_hl8(x, W_in, W_h, W_out):
    import ml_dtypes
    e4 = ml_dtypes.float8_e4m3fn

    def q8(a):
        return np.asarray(a, e4)

    def d8(a):
        return np.asarray(a, np.float32)

    xT = np.transpose(np.asarray(x, np.float32), (0, 2, 1))  # [B, D, T]
    x_hi = q8(xT)
    x_lo = q8((xT - d8(x_hi)) * 16.0)
    hi_r = x_hi.reshape(B, ND, 128, T)
    lo_r = x_lo.reshape(B, ND, 128, T)
    # planes: hi-d0, lo-d0, hi-d1, lo-d1
    xq = np.stack([hi_r[:, 0], lo_r[:, 0], hi_r[:, 1], lo_r[:, 1]], axis=2)
    xq = np.ascontiguousarray(xq)  # [B, 128, 4, T]

    wT = np.asarray(W_in, np.float32).T * SWI  # [D, H]
    W_hi = q8(wT)
    W_lo = q8((wT - d8(W_hi)) * 16.0)
    wa = np.ascontiguousarray(
        np.stack([W_hi[:128], q8(d8(W_hi[:128]) / 16.0)], axis=1))
    wb = np.ascontiguousarray(
        np.stack([W_hi[128:], q8(d8(W_hi[128:]) / 16.0)], axis=1))
    wc = np.ascontiguousarray(
        np.stack([q8(d8(W_lo[:128]) / 16.0), q8(d8(W_lo[128:]) / 16.0)], axis=1))

    w_outT = np.ascontiguousarray(
        np.asarray(W_out, np.float32).T).astype(ml_dtypes.bfloat16)
    d = np.ascontiguousarray(np.diagonal(np.asarray(W_h, np.float32)))
    dcols = np.ascontiguousarray(d.reshape(NH, 128).T, dtype=np.float32)

    in_maps = []
    for c in range(NCORES):
        in_maps.append({
            "xq": np.ascontiguousarray(xq[c * BPC:(c + 1) * BPC]),
            "wa": wa, "wb": wb, "wc": wc,
            "w_outT": w_outT,
            "dcols": dcols,
        })
    return in_maps


def _prep_inputs(x, W_in, W_h, W_out, mode: str):
    if mode == "hl8":
        return _prep_inputs_hl8(x, W_in, W_h, W_out)
    npdt = np.float32
    if mode == "bf16":
        import ml_dtypes
        npdt = ml_dtypes.bfloat16
    xT = np.ascontiguousarray(np.transpose(np.asarray(x, np.float32), (0, 2, 1))).astype(npdt)
    w_inT = np.ascontiguousarray(np.asarray(W_in, np.float32).T).astype(npdt)
    w_outT = np.ascontiguousarray(np.asarray(W_out, np.float32).T).astype(npdt)
    d = np.ascontiguousarray(np.diagonal(np.asarray(W_h, np.float32)))
    dcols = np.ascontiguousarray(d.reshape(NH, 128).T, dtype=np.float32)
    in_maps = []
    for c in range(NCORES):
        in_maps.append({
            "xT": np.ascontiguousarray(xT[c * BPC:(c + 1) * BPC]),
            "w_inT": w_inT,
            "w_outT": w_outT,
            "dcols": dcols,
        })
    return in_maps


def _get_nc(mode: str = MODE_DEFAULT):
    key = (mode, tuple(sorted(CFG.items())))
    if key not in _cache:
        _cache[key] = _build(mode)
    return _cache[key]


def _run(x, W_in, W_h, W_out, mode: str = MODE_DEFAULT, **spmd_kwargs):
    nc = _get_nc(mode)
    in_maps = _prep_inputs(x, W_in, W_h, W_out, mode)
    res = run_bass_kernel_spmd(nc, in_maps, list(range(NCORES)), **spmd_kwargs)
    scale = SWI if mode == "hl8" else 1.0
    parts = [np.transpose(np.asarray(res.results[c]["out"]).astype(np.float32),
                          (0, 2, 1)) / scale
             for c in range(NCORES)]
    full = np.concatenate(parts, axis=0).astype(np.float32)
    return full, res


def kernel(x, W_in, W_h, W_out):
    out, _ = _run(x, W_in, W_h, W_out)
    return out
